# revision 25
# baseline (speedup 1.0000x reference)
"""Multi-head attention (2D-RoPE, masked softmax) on 8 Trainium2 NeuronCores.

Sharding: 4 head-groups (3 heads each) x 2 query-halves (1160 rows each).
Each core computes full attention for its 3 heads over its 1160 query rows
against all 2320 keys, plus its share of the output projection; the host
sums the 8 partial projections and adds the (folded) biases.

v3 design notes:
  - Interleaved head-dim basis: rows [hA d0:32 | hB d0:32 | hA d32:64 |
    hB d32:64]. rotate_half becomes a uniform +-64 partition shift, so a
    2-head rope tile is copy + 3 muls + 1-4 adds on DVE instead of 8 ops.
    Heads (0,1) pack into one K tile (kt01, no zero padding needed -- the
    moving-side qt_h is zero-padded per head instead); head 2 packs its K
    and Q into one projection matmul (rows K|Q), valid since the permuted
    x layout puts this core's own query tokens first.
  - QKV biases fold into the projection matmuls as a K=1 matmul against a
    constant ones-row (stationary = bias row), freeing ScalarE entirely
    for exp; V masking runs on ScalarE as activation(Copy, scale=mask)
    (mask is per-partition there), freeing DVE for rope.
  - Output projection contracts head pairs: ctxn01 holds heads 0,1 in
    rows 0:64/64:128 (no zero padding), head 2 in ctxn2.
  - Emission order: g1 K|Q tiles -> all V tiles -> h2's three attention
    blocks (g0 K/Q tiles woven into their slots) -> h0/h1 blocks with
    deferred 1/Z chains and the previous L-tile's projections woven in.
  - DMA: few large 3D-AP transfers spread across sync/scalar/gpsimd
    queues; ~130 dummy matmuls hold the PE clock at 2.4 GHz through the
    DMA window; partial projections DMA out as bf16.
"""
import sys
if '/opt/trn_rl_repo' not in sys.path:
    sys.path.insert(0, '/opt/trn_rl_repo')
import numpy as np

SEQ, E, NH, D = 2320, 768, 12, 64
GRID, TASK = 48, 16
SQ = SEQ // 2           # query rows per core
HG = 3                  # heads per core
SCALE = D ** -0.5
EC = 6                  # embed chunks of 128
L_TILES = [(0, 512), (512, 512), (1024, 136)]
N_TILES = [(0, 512), (512, 512), (1024, 512), (1536, 512), (2048, 272)]
MC = [(i * 128, min(128, SEQ - i * 128)) for i in range(19)]
PT = [(i * 128, min(128, SQ - i * 128)) for i in range(10)]
WARM = 75               # PE warmup matmuls (N=128 each, ~128ns cold)

_prog = None


def _build():
    from contextlib import ExitStack
    import concourse.mybir as mybir
    import concourse.tile as tile
    from concourse import bacc

    F32, F32R = mybir.dt.float32, mybir.dt.float32r
    BF16 = mybir.dt.bfloat16
    AF = mybir.ActivationFunctionType

    nc = bacc.Bacc('TRN2', target_bir_lowering=False, debug=False, num_devices=8)
    dp = nc.declare_dram_parameter
    xt_d = dp("xt", [4, E, 580], BF16, isOutput=False)
    wk01_d = dp("wk01", [E, 128], BF16, isOutput=False)
    wq01_d = dp("wq01", [E, 128], BF16, isOutput=False)
    wkq2_d = dp("wkq2", [E, 128], BF16, isOutput=False)
    wv_d = dp("wv", [E, 192], BF16, isOutput=False)
    wp01_d = dp("wp01", [128, E], F32R, isOutput=False)
    wp2_d = dp("wp2", [64, E], F32R, isOutput=False)
    bias_d = dp("bias", [1, 3, 128], BF16, isOutput=False)
    cil_d = dp("cil", [128, SEQ], BF16, isOutput=False)
    sil_d = dp("sil", [128, SEQ], BF16, isOutput=False)
    mk_d = dp("mk", [128, 19], F32, isOutput=False)
    on_d = dp("ones64", [1, 64], F32R, isOutput=False)
    onr_d = dp("onesrow", [1, SEQ], BF16, isOutput=False)
    out_d = dp("pout", [SQ, E], BF16, isOutput=True)

    with tile.TileContext(nc) as tc:
        with (
            tc.tile_pool(name="long", bufs=1) as lp,
            tc.tile_pool(name="zp", bufs=2) as zp,
        ):
            kt01 = lp.tile([128, SEQ], BF16, tag="kt01")
            kt2 = lp.tile([128, SEQ], BF16, tag="kt2")
            qt_h = [lp.tile([128, SQ], BF16, tag=f"qt{h}", name=f"qt{h}")
                    for h in range(HG)]
            rk01 = lp.tile([128, SEQ], BF16, tag="rk01")
            rkq2 = lp.tile([128, SEQ], BF16, tag="rkq2")
            rq01 = lp.tile([128, SQ], BF16, tag="rq01")
            v_all = lp.tile([128, 19, HG, 65], BF16, tag="v_all")
            wp01_sb = lp.tile([128, E], F32R, tag="wp01")
            wp2_sb = lp.tile([128, E], F32R, tag="wp2")
            ctxn01 = lp.tile([128, SQ], F32R, tag="ctxn01")
            ctxn2 = lp.tile([128, SQ], F32R, tag="ctxn2")
            ones64 = lp.tile([1, 64], F32R, tag="ones64")
            ones_row = lp.tile([1, SEQ], BF16, tag="ones_row")
            bias_sb = lp.tile([1, 3, 128], BF16, tag="bias")
            mk_sb = lp.tile([128, 19], F32, tag="mk")
            cil_sb = lp.tile([128, SEQ], BF16, tag="cil")
            sil_sb = lp.tile([128, SEQ], BF16, tag="sil")
            warm_sb = lp.tile([128, 128], BF16, tag="warm")

            # ---- input DMA: few big transfers, spread across queues ----
            nc.gpsimd.memset(warm_sb[:], 0.0)
            nc.gpsimd.dma_start(cil_sb[:], cil_d[:])
            nc.gpsimd.dma_start(sil_sb[:], sil_d[:])
            nc.gpsimd.dma_start(mk_sb[:], mk_d[:])
            nc.gpsimd.dma_start(ones64[:], on_d[:])
            nc.gpsimd.dma_start(ones_row[:], onr_d[:])
            nc.gpsimd.memset(kt2[:], 0.0)
            for h in range(HG):
                nc.gpsimd.memset(qt_h[h][:], 0.0)
            nc.gpsimd.memset(ctxn2[64:128, :].bitcast(F32), 0.0)
            nc.gpsimd.memset(wp2_sb[64:128, :].bitcast(F32), 0.0)

            es = ExitStack()
            p12 = es.enter_context(tc.tile_pool(name="p12", bufs=1))
            pvst = ExitStack()
            pkp0 = pvst.enter_context(tc.tile_pool(name="pkA", bufs=3, space="PSUM"))
            pkp = [(pkp0, "pk")]
            vvp = [None]

            wk_sb = p12.tile([128, EC, 128], BF16, tag="wk")
            wq_sb = p12.tile([128, EC, 128], BF16, tag="wq")
            wkq_sb = p12.tile([128, EC, 128], BF16, tag="wkq")
            wv_sb = p12.tile([128, EC, 192], BF16, tag="wv")
            xt = p12.tile([128, EC, SEQ], BF16, tag="xt")
            # sync queue: big operands in first-use order
            nc.sync.dma_start(wkq_sb[:, :, :],
                              wkq2_d.rearrange("(c p) n -> p c n", p=128))
            nc.sync.dma_start(xt[:, :, 0:580],
                              xt_d[0].rearrange("(c p) n -> p c n", p=128))
            nc.sync.dma_start(xt[:, :, 580:1160],
                              xt_d[1].rearrange("(c p) n -> p c n", p=128))
            nc.sync.dma_start(xt[:, :, 1160:1740],
                              xt_d[2].rearrange("(c p) n -> p c n", p=128))
            # scalar queue: small/late tensors
            nc.scalar.dma_start(bias_sb[:], bias_d[:])
            nc.scalar.dma_start(wv_sb[:, :, :],
                                wv_d.rearrange("(c p) n -> p c n", p=128))
            nc.scalar.dma_start(xt[:, :, 1740:2320],
                                xt_d[3].rearrange("(c p) n -> p c n", p=128))
            nc.scalar.dma_start(wk_sb[:, :, :],
                                wk01_d.rearrange("(c p) n -> p c n", p=128))
            nc.scalar.dma_start(wq_sb[:, :, :],
                                wq01_d.rearrange("(c p) n -> p c n", p=128))
            nc.scalar.dma_start(wp01_sb[:], wp01_d[:])
            nc.scalar.dma_start(wp2_sb[0:64, :], wp2_d[:])

            # ---- PE warmup: hold HAM at 2.4GHz through the DMA window
            warm_ps = pkp0.tile([128, 512], F32, tag="pk", name="warm_ps")
            for _ in range(WARM):
                nc.tensor.matmul(warm_ps[0:128, 0:128], warm_sb[:],
                                 warm_sb[:], start=True, stop=True)

            # ---- QKV projection tiles (bias folded as K=1 matmul) ----
            def qk_mm(w_sb, bidx, off, n):
                pool, tag = pkp[0]
                ps = pool.tile([128, 512], F32, tag=tag, name="pk")
                for c in range(EC):
                    nc.tensor.matmul(
                        ps[0:128, 0:n], w_sb[:, c, :], xt[:, c, off:off + n],
                        start=(c == 0), stop=False)
                nc.tensor.matmul(
                    ps[0:128, 0:n], bias_sb[0:1, bidx, :],
                    ones_row[0:1, off:off + n], start=False, stop=True)
                return ps

            # ---- rope in the interleaved basis ----
            def rope_il(ps, raw, off, n, kind):
                if kind == "kq2":
                    nc.scalar.copy(raw[0:128, off:off + n], ps[0:128, 0:n])
                else:
                    nc.vector.tensor_copy(raw[0:128, off:off + n], ps[0:128, 0:n])
                t1 = zp.tile([128, 512], BF16, tag="rt1", name="rt1", bufs=3)
                t2 = zp.tile([128, 512], BF16, tag="rt2", name="rt2", bufs=3)
                nc.vector.tensor_mul(
                    t1[0:128, 0:n], raw[0:128, off:off + n],
                    cil_sb[0:128, off:off + n])
                nc.vector.tensor_mul(
                    t2[0:64, 0:n], raw[64:128, off:off + n],
                    sil_sb[64:128, off:off + n])
                nc.vector.tensor_mul(
                    t2[64:128, 0:n], raw[0:64, off:off + n],
                    sil_sb[0:64, off:off + n])
                if kind == "k01":
                    nc.vector.tensor_add(
                        kt01[0:128, off:off + n], t1[0:128, 0:n], t2[0:128, 0:n])
                elif kind == "q01":
                    nc.vector.tensor_add(
                        qt_h[0][0:32, off:off + n], t1[0:32, 0:n], t2[0:32, 0:n])
                    nc.vector.tensor_add(
                        qt_h[0][64:96, off:off + n], t1[64:96, 0:n], t2[64:96, 0:n])
                    nc.vector.tensor_add(
                        qt_h[1][32:64, off:off + n], t1[32:64, 0:n], t2[32:64, 0:n])
                    nc.vector.tensor_add(
                        qt_h[1][96:128, off:off + n], t1[96:128, 0:n],
                        t2[96:128, 0:n])
                else:  # kq2: K rows stay, Q rows shift up 32
                    nc.vector.tensor_add(
                        kt2[0:32, off:off + n], t1[0:32, 0:n], t2[0:32, 0:n])
                    nc.vector.tensor_add(
                        kt2[64:96, off:off + n], t1[64:96, 0:n], t2[64:96, 0:n])
                    nq = min(n, SQ - off)
                    if nq > 0:
                        nc.vector.tensor_add(
                            qt_h[2][0:32, off:off + nq], t1[32:64, 0:nq],
                            t2[32:64, 0:nq])
                        nc.vector.tensor_add(
                            qt_h[2][64:96, off:off + nq], t1[96:128, 0:nq],
                            t2[96:128, 0:nq])

            def k01_job(off, n):
                rope_il(qk_mm(wk_sb, 0, off, n), rk01, off, n, "k01")

            def q01_job(off, n):
                rope_il(qk_mm(wq_sb, 1, off, n), rq01, off, n, "q01")

            def kq2_job(off, n):
                rope_il(qk_mm(wkq_sb, 2, off, n), rkq2, off, n, "kq2")

            def v_tile(i):
                off, m = MC[i]
                pv = vvp[0].tile([128, 512], F32, tag="pv", name="pv")
                for c in range(EC):
                    nc.tensor.matmul(
                        pv[0:m, 0:192], xt[:, c, off:off + m], wv_sb[:, c, :],
                        start=(c == 0), stop=(c == EC - 1))
                if i % 2 == 0:
                    nc.scalar.activation(
                        v_all[0:m, i, :, 0:64],
                        pv[0:m, 0:192].rearrange("p (h d) -> p h d", h=HG),
                        AF.Copy, bias=0.0, scale=mk_sb[0:m, i:i + 1])
                else:
                    nc.vector.tensor_mul(
                        v_all[0:m, i, :, 0:64],
                        pv[0:m, 0:192].rearrange("p (h d) -> p h d", h=HG),
                        mk_sb[0:m, i:i + 1].to_broadcast([m, HG, 64]))

            # ones-column plane of v_all (static: per-key mask value)
            for h in range(HG):
                nc.vector.tensor_copy(
                    v_all[:, :, h, 64:65],
                    mk_sb[:, 0:19].to_broadcast([128, 19, 1]))
            # phase A head: g1 K|Q tiles back-to-back (their ropes gate the
            # first exp); V tiles are woven into the first attention block's
            # slots so the exp stream starts as soon as kt2 is roped
            for (off, n) in N_TILES:
                kq2_job(off, n)
            pvst.close()

            # fillers woven into h2's attention blocks
            fillers = [("k", off, n) for (off, n) in N_TILES]
            fillers += [("q", off, n) for (off, n) in L_TILES]

            # ---- attention ----
            with tc.tile_pool(name="ep", bufs=3) as ep, \
                 tc.tile_pool(name="op", bufs=2) as op, \
                 tc.tile_pool(name="rzp", bufs=4) as rzp, \
                 tc.tile_pool(name="ps3", bufs=2, space="PSUM") as ps3, \
                 tc.tile_pool(name="pc3", bufs=1, space="PSUM") as pc3:
                pkp[0] = (pc3, "pv")
                vvp[0] = pc3
                PROJ_OF_LT = {0: PT[0:4], 1: PT[4:8], 2: PT[8:10]}
                KT_OF_H = {0: kt01, 1: kt01, 2: kt2}
                CTX_OF_H = {0: (ctxn01, 0), 1: (ctxn01, 64), 2: (ctxn2, 0)}
                pending = []
                proj_q = []
                vq = list(range(len(MC)))

                def proj_slice(toff, tm):
                    outsb = op.tile([128, E], BF16, tag="outsb", name="outsb")
                    for half in range(2):
                        hs = half * 384
                        pp = pc3.tile([128, 512], F32, tag="pp", name="pp")
                        nc.tensor.matmul(
                            pp[0:tm, 0:384], ctxn01[0:128, toff:toff + tm],
                            wp01_sb[0:128, hs:hs + 384], start=True, stop=False)
                        nc.tensor.matmul(
                            pp[0:tm, 0:384], ctxn2[0:128, toff:toff + tm],
                            wp2_sb[0:128, hs:hs + 384], start=False, stop=True)
                        nc.vector.tensor_copy(outsb[0:tm, hs:hs + 384],
                                              pp[0:tm, 0:384])
                    nc.gpsimd.dma_start(out_d[toff:toff + tm, :], outsb[0:tm, :])

                def finish_tile(z):
                    zrow, ctxu, ctxap2, base, loff2, ln2 = z
                    zscr = zp.tile([1, 512], F32, tag="zscr", name="zscr")
                    rzf = zp.tile([1, 512], F32, tag="rzf", name="rzf")
                    nc.vector.reciprocal_approx_accurate(
                        rzf[0:1, 0:ln2], zrow[0:1, 0:ln2], zscr[0:1, 0:ln2])
                    rzr = zp.tile([1, 512], F32R, tag="rzr", name="rzr")
                    nc.vector.tensor_copy(rzr[0:1, 0:ln2], rzf[0:1, 0:ln2])
                    przb = pc3.tile([128, 512], F32, tag="pp", name="przb")
                    nc.tensor.matmul(
                        przb[0:64, 0:ln2], ones64[:], rzr[0:1, 0:ln2],
                        start=True, stop=True)
                    rzb = rzp.tile([64, 512], F32, tag="rzb", name="rzb")
                    nc.vector.tensor_copy(rzb[:, 0:ln2], przb[0:64, 0:ln2])
                    nc.vector.tensor_mul(
                        ctxap2[base:base + 64, loff2:loff2 + ln2],
                        ctxu[0:64, 0:ln2], rzb[0:64, 0:ln2])

                BLOCKS = [(2, 0), (2, 1), (2, 2),
                          (0, 0), (1, 0), (0, 1), (1, 1), (0, 2), (1, 2)]
                for bn, (h, lt_i) in enumerate(BLOCKS):
                    loff, ln = L_TILES[lt_i]
                    ktap, qtap = KT_OF_H[h], qt_h[h]
                    ctxap, cbase = CTX_OF_H[h]
                    pctx = pc3.tile([65, 512], F32, tag="pctx")
                    PAIRS = [(i, i + 1) if i + 1 < len(MC) else (i,)
                             for i in range(0, len(MC), 2)]
                    exs = {}

                    def scores_exp(p):
                        chunks = PAIRS[p]
                        ps = ps3.tile([128, 1024], F32, tag="ps", name="ps")
                        for j, i in enumerate(chunks):
                            moff, m = MC[i]
                            nc.tensor.matmul(
                                ps[0:m, j * 512:j * 512 + ln],
                                ktap[0:128, moff:moff + m],
                                qtap[0:128, loff:loff + ln],
                                start=True, stop=True)
                        ex = ep.tile([128, 1024], BF16, tag="ex", name="ex")
                        m0 = MC[chunks[0]][1]
                        if len(chunks) == 2:
                            nc.scalar.activation(
                                ex[0:m0, 0:2 * ln].rearrange(
                                    "p (two n) -> p two n", two=2),
                                ps[0:m0, :].rearrange(
                                    "p (two n) -> p two n", two=2)[:, :, 0:ln],
                                AF.Exp, bias=0.0, scale=SCALE)
                        else:
                            nc.scalar.activation(
                                ex[0:m0, 0:ln], ps[0:m0, 0:ln], AF.Exp,
                                bias=0.0, scale=SCALE)
                        exs[p] = ex

                    def ctx_mm(p):
                        ex = exs.pop(p)
                        for j, i in enumerate(PAIRS[p]):
                            moff, m = MC[i]
                            nc.tensor.matmul(
                                pctx[:, 0:ln], v_all[0:m, i, h, :],
                                ex[0:m, j * ln:j * ln + ln],
                                start=(i == 0), stop=(i == len(MC) - 1))

                    for p in range(len(PAIRS) + 2):
                        if p < len(PAIRS):
                            scores_exp(p)
                        if bn == 0 and vq:
                            v_tile(vq.pop(0))
                            if vq:
                                v_tile(vq.pop(0))
                        if p in (2, 6) and pending:
                            finish_tile(pending.pop(0))
                        if bn in (1, 2) and p in (1, 3, 5, 7) and fillers:
                            kind, foff, fn = fillers.pop(0)
                            if kind == "k":
                                k01_job(foff, fn)
                            else:
                                q01_job(foff, fn)
                        if bn >= 5 and p in (5, 8) and proj_q:
                            proj_slice(*proj_q.pop(0))
                        if p >= 2:
                            ctx_mm(p - 2)

                    zrow = zp.tile([1, 512], F32, tag="zrow", name="zrow",
                                   bufs=4)
                    nc.vector.tensor_copy(zrow[0:1, 0:ln], pctx[64:65, 0:ln])
                    ctxu = rzp.tile([64, 512], F32, tag="ctxu", name="ctxu")
                    nc.vector.tensor_copy(ctxu[:, 0:ln], pctx[0:64, 0:ln])
                    pending.append((zrow, ctxu, ctxap, cbase, loff, ln))
                    if bn == 4:
                        proj_q.extend(PROJ_OF_LT[0])
                    elif bn == 6:
                        proj_q.extend(PROJ_OF_LT[1])
                    elif bn == 8:
                        proj_q.extend(PROJ_OF_LT[2])
                # tail: remaining chains, then projections
                while pending:
                    finish_tile(pending.pop(0))
                while proj_q:
                    proj_slice(*proj_q.pop(0))
            es.close()

    nc.finalize()
    return nc


def _rope_tables():
    dim = D // 2
    freqs = 1.0 / 10000 ** (np.arange(0, dim, 2, dtype=np.float64) / dim)
    t = np.arange(GRID, dtype=np.float64)
    f = np.repeat(np.outer(t, freqs), 2, axis=-1)                  # [48, 32]
    fr = np.broadcast_to(f[:, None, :], (GRID, GRID, dim))
    fc = np.broadcast_to(f[None, :, :], (GRID, GRID, dim))
    full = np.concatenate([fr, fc], axis=-1).reshape(GRID * GRID, D)
    cos = np.ones((SEQ, D), np.float64)
    sin = np.zeros((SEQ, D), np.float64)
    cos[TASK:] = np.cos(full)
    sin[TASK:] = np.sin(full)
    return cos.astype(np.float32), sin.astype(np.float32)


# interleaved basis: row r <-> (slot=(r//32)%2, d=(r%32)+32*(r//64))
_R = np.arange(128)
_DSEL = (_R % 32) + 32 * (_R // 64)
_HSEL = (_R // 32) % 2


def _core_inputs(x, mask, Wqkv, Wproj, bqkv, cos, sin, g, s):
    xT = x.T  # [768, 2320]
    if s == 0:
        perm = None
        xt = np.ascontiguousarray(xT)
    else:
        perm = np.concatenate([np.arange(SQ, SEQ), np.arange(0, SQ)])
        xt = np.ascontiguousarray(np.concatenate([xT[:, SQ:], xT[:, :SQ]], axis=1))
    r0 = 192 * g
    wq = Wqkv[r0:r0 + 192, :].T                  # [768, 192]
    wk = Wqkv[768 + r0:768 + r0 + 192, :].T
    wv = np.ascontiguousarray(Wqkv[1536 + r0:1536 + r0 + 192, :].T)
    # interleaved-basis weight column orders
    wk01 = wk[:, _HSEL * 64 + _DSEL]                         # heads 0,1
    wq01 = wq[:, _HSEL * 64 + _DSEL]
    wkq2 = np.where(_HSEL[None, :] == 0,
                    wk[:, 128 + _DSEL], wq[:, 128 + _DSEL])  # head2 K|Q
    bq = bqkv[r0:r0 + 192]
    bk = bqkv[768 + r0:768 + r0 + 192]
    bias = np.zeros((1, 3, 128), np.float32)
    bias[0, 0] = bk[_HSEL * 64 + _DSEL]
    bias[0, 1] = bq[_HSEL * 64 + _DSEL]
    bias[0, 2] = np.where(_HSEL == 0, bk[128 + _DSEL], bq[128 + _DSEL])
    wp01 = np.ascontiguousarray(Wproj[:, r0:r0 + 128].T)     # [128, 768]
    wp2 = np.ascontiguousarray(Wproj[:, r0 + 128:r0 + 192].T)  # [64, 768]
    cosT, sinT = cos.T, sin.T  # [64, S]
    cil = cosT[_DSEL]                                        # [128, S]
    # sine table stored at the SOURCE rows so both tensor_tensor inputs
    # share a base partition: rows 64:128 feed t2[0:64] (sign -), etc.
    sil = np.empty((128, SEQ), np.float32)
    sil[64:128] = -sinT[_DSEL[0:64]]
    sil[0:64] = sinT[_DSEL[64:128]]
    if perm is not None:
        cil = cil[:, perm]
        sil = sil[:, perm]
    mk = mask.astype(np.float32)
    if perm is not None:
        mk = mk[perm]
    mk = np.concatenate([mk, np.zeros(19 * 128 - SEQ, np.float32)])
    mk = np.ascontiguousarray(mk.reshape(19, 128).T)
    import ml_dtypes
    bf = ml_dtypes.bfloat16
    return {
        "xt": np.ascontiguousarray(
            np.stack([xt[:, i * 580:(i + 1) * 580] for i in range(4)])
        ).astype(bf),
        "wk01": np.ascontiguousarray(wk01).astype(bf),
        "wq01": np.ascontiguousarray(wq01).astype(bf),
        "wkq2": np.ascontiguousarray(wkq2).astype(bf),
        "wv": wv.astype(bf),
        "wp01": wp01, "wp2": wp2,
        "bias": bias.astype(bf),
        "cil": np.ascontiguousarray(cil).astype(bf),
        "sil": np.ascontiguousarray(sil).astype(bf),
        "mk": np.ascontiguousarray(mk),
        "ones64": np.ones((1, 64), np.float32),
        "onesrow": np.ones((1, SEQ), ml_dtypes.bfloat16),
    }


def _run(x, mask, Wqkv, bqkv, Wproj, bproj, trace=False):
    global _prog
    from concourse.bass_utils import run_bass_kernel_spmd
    if _prog is None:
        _prog = _build()
    x = np.asarray(x, np.float32)
    mask = np.asarray(mask)
    Wqkv = np.asarray(Wqkv, np.float32)
    bqkv = np.asarray(bqkv, np.float32)
    Wproj = np.asarray(Wproj, np.float32)
    bproj = np.asarray(bproj, np.float32)
    cos, sin = _rope_tables()
    in_maps = [
        _core_inputs(x, mask, Wqkv, Wproj, bqkv, cos, sin, core // 2, core % 2)
        for core in range(8)
    ]
    res = run_bass_kernel_spmd(_prog, in_maps, list(range(8)), trace=trace)
    acc = np.zeros((SEQ, E), np.float64)
    for core in range(8):
        s = core % 2
        acc[SQ * s:SQ * (s + 1)] += res.results[core]["pout"].astype(np.float64)
    bias_row = bproj.astype(np.float64) + Wproj.astype(np.float64) @ \
        bqkv[1536:2304].astype(np.float64)
    acc += bias_row
    return acc.astype(np.float32), res


def kernel(x, mask, Wqkv, bqkv, Wproj, bproj):
    out, _ = _run(x, mask, Wqkv, bqkv, Wproj, bproj, trace=False)
    return out


# revision 27
# speedup vs baseline: 1.2147x; 1.2147x over previous
"""Multi-head attention (2D-RoPE, masked softmax) on 8 Trainium2 NeuronCores.

Sharding: 4 head-groups (3 heads each) x 2 query-halves (1160 rows each).
Each core computes full attention for its 3 heads over its 1160 query rows
against all 2320 keys, plus its share of the output projection; the host
sums the 8 partial projections and adds the (folded) biases.

v3 design notes:
  - Interleaved head-dim basis: rows [hA d0:32 | hB d0:32 | hA d32:64 |
    hB d32:64]. rotate_half becomes a uniform +-64 partition shift, so a
    2-head rope tile is copy + 3 muls + 1-4 adds on DVE instead of 8 ops.
    Heads (0,1) pack into one K tile (kt01, no zero padding needed -- the
    moving-side qt_h is zero-padded per head instead); head 2 packs its K
    and Q into one projection matmul (rows K|Q), valid since the permuted
    x layout puts this core's own query tokens first.
  - QKV biases fold into the projection matmuls as a K=1 matmul against a
    constant ones-row (stationary = bias row), freeing ScalarE entirely
    for exp; V masking runs on ScalarE as activation(Copy, scale=mask)
    (mask is per-partition there), freeing DVE for rope.
  - Output projection contracts head pairs: ctxn01 holds heads 0,1 in
    rows 0:64/64:128 (no zero padding), head 2 in ctxn2.
  - Emission order: g1 K|Q tiles -> all V tiles -> h2's three attention
    blocks (g0 K/Q tiles woven into their slots) -> h0/h1 blocks with
    deferred 1/Z chains and the previous L-tile's projections woven in.
  - DMA: few large 3D-AP transfers spread across sync/scalar/gpsimd
    queues; ~130 dummy matmuls hold the PE clock at 2.4 GHz through the
    DMA window; partial projections DMA out as bf16.
"""
import sys
if '/opt/trn_rl_repo' not in sys.path:
    sys.path.insert(0, '/opt/trn_rl_repo')
import numpy as np

SEQ, E, NH, D = 2320, 768, 12, 64
GRID, TASK = 48, 16
SQ = SEQ // 2           # query rows per core
HG = 3                  # heads per core
SCALE = D ** -0.5
EC = 6                  # embed chunks of 128
L_TILES = [(0, 512), (512, 512), (1024, 136)]
N_TILES = [(0, 512), (512, 512), (1024, 512), (1536, 512), (2048, 272)]
MC = [(i * 128, min(128, SEQ - i * 128)) for i in range(19)]
PT = [(i * 128, min(128, SQ - i * 128)) for i in range(10)]
WARM = 75               # PE warmup matmuls (N=128 each, ~128ns cold)

_prog = None


def _build():
    from contextlib import ExitStack
    import concourse.mybir as mybir
    import concourse.tile as tile
    from concourse import bacc

    F32, F32R = mybir.dt.float32, mybir.dt.float32r
    BF16 = mybir.dt.bfloat16
    AF = mybir.ActivationFunctionType

    nc = bacc.Bacc('TRN2', target_bir_lowering=False, debug=False, num_devices=8)
    dp = nc.declare_dram_parameter
    xt_d = dp("xt", [4, E, 580], BF16, isOutput=False)
    wk01_d = dp("wk01", [E, 128], BF16, isOutput=False)
    wq01_d = dp("wq01", [E, 128], BF16, isOutput=False)
    wkq2_d = dp("wkq2", [E, 128], BF16, isOutput=False)
    wv_d = dp("wv", [E, 192], BF16, isOutput=False)
    wp01_d = dp("wp01", [128, E], F32R, isOutput=False)
    wp2_d = dp("wp2", [64, E], F32R, isOutput=False)
    bias_d = dp("bias", [1, 3, 128], BF16, isOutput=False)
    cil_d = dp("cil", [128, SEQ], BF16, isOutput=False)
    sil_d = dp("sil", [128, SEQ], BF16, isOutput=False)
    mk_d = dp("mk", [128, 19], F32, isOutput=False)
    on_d = dp("ones64", [1, 64], F32R, isOutput=False)
    onr_d = dp("onesrow", [1, SEQ], BF16, isOutput=False)
    out_d = dp("pout", [SQ, E], BF16, isOutput=True)

    with tile.TileContext(nc) as tc:
        with (
            tc.tile_pool(name="long", bufs=1) as lp,
            tc.tile_pool(name="zp", bufs=2) as zp,
        ):
            kt01 = lp.tile([128, SEQ], BF16, tag="kt01")
            kt2 = lp.tile([128, SEQ], BF16, tag="kt2")
            qt_h = [lp.tile([128, SQ], BF16, tag=f"qt{h}", name=f"qt{h}")
                    for h in range(HG)]
            rk01 = lp.tile([128, SEQ], BF16, tag="rk01")
            rkq2 = lp.tile([128, SEQ], BF16, tag="rkq2")
            rq01 = lp.tile([128, SQ], BF16, tag="rq01")
            v_all = lp.tile([128, 19, HG, 65], BF16, tag="v_all")
            wp01_sb = lp.tile([128, E], F32R, tag="wp01")
            wp2_sb = lp.tile([128, E], F32R, tag="wp2")
            ctxn01 = lp.tile([128, SQ], F32R, tag="ctxn01")
            ctxn2 = lp.tile([128, SQ], F32R, tag="ctxn2")
            ones64 = lp.tile([1, 64], F32R, tag="ones64")
            ones_row = lp.tile([1, SEQ], BF16, tag="ones_row")
            bias_sb = lp.tile([1, 3, 128], BF16, tag="bias")
            mk_sb = lp.tile([128, 19], F32, tag="mk")
            cil_sb = lp.tile([128, SEQ], BF16, tag="cil")
            sil_sb = lp.tile([128, SEQ], BF16, tag="sil")
            warm_sb = lp.tile([128, 128], BF16, tag="warm")

            # ---- input DMA: few big transfers, spread across queues ----
            nc.gpsimd.memset(warm_sb[:], 0.0)
            nc.gpsimd.dma_start(cil_sb[:], cil_d[:])
            nc.gpsimd.dma_start(sil_sb[:], sil_d[:])
            nc.gpsimd.dma_start(mk_sb[:], mk_d[:])
            nc.gpsimd.dma_start(ones64[:], on_d[:])
            nc.gpsimd.dma_start(ones_row[:], onr_d[:])
            nc.gpsimd.memset(kt2[:], 0.0)
            for h in range(HG):
                nc.gpsimd.memset(qt_h[h][:], 0.0)
            nc.gpsimd.memset(ctxn2[64:128, :].bitcast(F32), 0.0)
            nc.gpsimd.memset(wp2_sb[64:128, :].bitcast(F32), 0.0)

            es = ExitStack()
            p12 = es.enter_context(tc.tile_pool(name="p12", bufs=1))
            pvst = ExitStack()
            pkp0 = pvst.enter_context(tc.tile_pool(name="pkA", bufs=3, space="PSUM"))
            pvp = pvst.enter_context(tc.tile_pool(name="pv", bufs=2, space="PSUM"))
            pkp = [pkp0]

            wk_sb = p12.tile([128, EC, 128], BF16, tag="wk")
            wq_sb = p12.tile([128, EC, 128], BF16, tag="wq")
            wkq_sb = p12.tile([128, EC, 128], BF16, tag="wkq")
            wv_sb = p12.tile([128, EC, 192], BF16, tag="wv")
            xt = p12.tile([128, EC, SEQ], BF16, tag="xt")
            # sync queue: big operands in first-use order
            nc.sync.dma_start(wkq_sb[:, :, :],
                              wkq2_d.rearrange("(c p) n -> p c n", p=128))
            nc.sync.dma_start(xt[:, :, 0:580],
                              xt_d[0].rearrange("(c p) n -> p c n", p=128))
            nc.sync.dma_start(xt[:, :, 580:1160],
                              xt_d[1].rearrange("(c p) n -> p c n", p=128))
            nc.sync.dma_start(xt[:, :, 1160:1740],
                              xt_d[2].rearrange("(c p) n -> p c n", p=128))
            # scalar queue: small/late tensors
            nc.scalar.dma_start(bias_sb[:], bias_d[:])
            nc.scalar.dma_start(wv_sb[:, :, :],
                                wv_d.rearrange("(c p) n -> p c n", p=128))
            nc.scalar.dma_start(xt[:, :, 1740:2320],
                                xt_d[3].rearrange("(c p) n -> p c n", p=128))
            nc.scalar.dma_start(wk_sb[:, :, :],
                                wk01_d.rearrange("(c p) n -> p c n", p=128))
            nc.scalar.dma_start(wq_sb[:, :, :],
                                wq01_d.rearrange("(c p) n -> p c n", p=128))
            nc.scalar.dma_start(wp01_sb[:], wp01_d[:])
            nc.scalar.dma_start(wp2_sb[0:64, :], wp2_d[:])

            # ---- PE warmup: hold HAM at 2.4GHz through the DMA window
            warm_ps = pkp0.tile([128, 512], F32, tag="warm", name="warm_ps")
            for _ in range(WARM):
                nc.tensor.matmul(warm_ps[0:128, 0:128], warm_sb[:],
                                 warm_sb[:], start=True, stop=True)

            # ---- QKV projection tiles (bias folded as K=1 matmul) ----
            def qk_mm(w_sb, bidx, off, n):
                ps = pkp[0].tile([128, 512], F32, tag="pk", name="pk")
                for c in range(EC):
                    nc.tensor.matmul(
                        ps[0:128, 0:n], w_sb[:, c, :], xt[:, c, off:off + n],
                        start=(c == 0), stop=False)
                nc.tensor.matmul(
                    ps[0:128, 0:n], bias_sb[0:1, bidx, :],
                    ones_row[0:1, off:off + n], start=False, stop=True)
                return ps

            # ---- rope in the interleaved basis ----
            def rope_il(ps, raw, off, n, kind):
                if kind == "kq2":
                    nc.scalar.copy(raw[0:128, off:off + n], ps[0:128, 0:n])
                else:
                    nc.vector.tensor_copy(raw[0:128, off:off + n], ps[0:128, 0:n])
                t1 = zp.tile([128, 512], BF16, tag="rt1", name="rt1", bufs=3)
                t2 = zp.tile([128, 512], BF16, tag="rt2", name="rt2", bufs=3)
                nc.vector.tensor_mul(
                    t1[0:128, 0:n], raw[0:128, off:off + n],
                    cil_sb[0:128, off:off + n])
                nc.vector.tensor_mul(
                    t2[0:64, 0:n], raw[64:128, off:off + n],
                    sil_sb[64:128, off:off + n])
                nc.vector.tensor_mul(
                    t2[64:128, 0:n], raw[0:64, off:off + n],
                    sil_sb[0:64, off:off + n])
                if kind == "k01":
                    nc.vector.tensor_add(
                        kt01[0:128, off:off + n], t1[0:128, 0:n], t2[0:128, 0:n])
                elif kind == "q01":
                    nc.vector.tensor_add(
                        qt_h[0][0:32, off:off + n], t1[0:32, 0:n], t2[0:32, 0:n])
                    nc.vector.tensor_add(
                        qt_h[0][64:96, off:off + n], t1[64:96, 0:n], t2[64:96, 0:n])
                    nc.vector.tensor_add(
                        qt_h[1][32:64, off:off + n], t1[32:64, 0:n], t2[32:64, 0:n])
                    nc.vector.tensor_add(
                        qt_h[1][96:128, off:off + n], t1[96:128, 0:n],
                        t2[96:128, 0:n])
                else:  # kq2: K rows stay, Q rows shift up 32
                    nc.vector.tensor_add(
                        kt2[0:32, off:off + n], t1[0:32, 0:n], t2[0:32, 0:n])
                    nc.vector.tensor_add(
                        kt2[64:96, off:off + n], t1[64:96, 0:n], t2[64:96, 0:n])
                    nq = min(n, SQ - off)
                    if nq > 0:
                        nc.vector.tensor_add(
                            qt_h[2][0:32, off:off + nq], t1[32:64, 0:nq],
                            t2[32:64, 0:nq])
                        nc.vector.tensor_add(
                            qt_h[2][64:96, off:off + nq], t1[96:128, 0:nq],
                            t2[96:128, 0:nq])

            def k01_job(off, n):
                rope_il(qk_mm(wk_sb, 0, off, n), rk01, off, n, "k01")

            def q01_job(off, n):
                rope_il(qk_mm(wq_sb, 1, off, n), rq01, off, n, "q01")

            def kq2_job(off, n):
                rope_il(qk_mm(wkq_sb, 2, off, n), rkq2, off, n, "kq2")

            def v_tile(i):
                off, m = MC[i]
                pv = pvp.tile([128, 192], F32, tag="pv", name="pv")
                for c in range(EC):
                    nc.tensor.matmul(
                        pv[0:m, :], xt[:, c, off:off + m], wv_sb[:, c, :],
                        start=(c == 0), stop=(c == EC - 1))
                if i % 2 == 0:
                    nc.scalar.activation(
                        v_all[0:m, i, :, 0:64],
                        pv[0:m, 0:192].rearrange("p (h d) -> p h d", h=HG),
                        AF.Copy, bias=0.0, scale=mk_sb[0:m, i:i + 1])
                else:
                    nc.vector.tensor_mul(
                        v_all[0:m, i, :, 0:64],
                        pv[0:m, 0:192].rearrange("p (h d) -> p h d", h=HG),
                        mk_sb[0:m, i:i + 1].to_broadcast([m, HG, 64]))

            # ones-column plane of v_all (static: per-key mask value)
            for h in range(HG):
                nc.vector.tensor_copy(
                    v_all[:, :, h, 64:65],
                    mk_sb[:, 0:19].to_broadcast([128, 19, 1]))
            # phase A head: g1 K|Q tiles back-to-back (their ropes gate the
            # first exp), then all V tiles (masking on DVE trails into the
            # first attention block). Warm fillers bridge the xt DMA gaps so
            # HAM never re-throttles mid-phase.
            for ti, (off, n) in enumerate(N_TILES):
                kq2_job(off, n)
                if ti < 2:
                    for _ in range(30):
                        nc.tensor.matmul(warm_ps[0:128, 0:128], warm_sb[:],
                                         warm_sb[:], start=True, stop=True)
            for i in range(len(MC)):
                v_tile(i)
            pvst.close()

            # fillers woven into h2's attention blocks
            fillers = [("k", off, n) for (off, n) in N_TILES]
            fillers += [("q", off, n) for (off, n) in L_TILES]

            # ---- attention ----
            with tc.tile_pool(name="ep", bufs=3) as ep, \
                 tc.tile_pool(name="op", bufs=2) as op, \
                 tc.tile_pool(name="rzp", bufs=4) as rzp, \
                 tc.tile_pool(name="ps3", bufs=2, space="PSUM") as ps3, \
                 tc.tile_pool(name="pc3", bufs=1, space="PSUM") as pc3:
                pkp[0] = pc3
                PROJ_OF_LT = {0: PT[0:4], 1: PT[4:8], 2: PT[8:10]}
                KT_OF_H = {0: kt01, 1: kt01, 2: kt2}
                CTX_OF_H = {0: (ctxn01, 0), 1: (ctxn01, 64), 2: (ctxn2, 0)}
                pending = []
                proj_q = []

                def proj_slice(toff, tm):
                    outsb = op.tile([128, E], BF16, tag="outsb", name="outsb")
                    for half in range(2):
                        hs = half * 384
                        pp = pc3.tile([128, 512], F32,
                                      tag=("pp" if half == 0 else "przb"),
                                      name="pp")
                        nc.tensor.matmul(
                            pp[0:tm, 0:384], ctxn01[0:128, toff:toff + tm],
                            wp01_sb[0:128, hs:hs + 384], start=True, stop=False)
                        nc.tensor.matmul(
                            pp[0:tm, 0:384], ctxn2[0:128, toff:toff + tm],
                            wp2_sb[0:128, hs:hs + 384], start=False, stop=True)
                        nc.vector.tensor_copy(outsb[0:tm, hs:hs + 384],
                                              pp[0:tm, 0:384])
                    nc.gpsimd.dma_start(out_d[toff:toff + tm, :], outsb[0:tm, :])

                def finish_tile(z):
                    zrow, ctxu, ctxap2, base, loff2, ln2 = z
                    zscr = zp.tile([1, 512], F32, tag="zscr", name="zscr")
                    rzf = zp.tile([1, 512], F32, tag="rzf", name="rzf")
                    nc.vector.reciprocal_approx_accurate(
                        rzf[0:1, 0:ln2], zrow[0:1, 0:ln2], zscr[0:1, 0:ln2])
                    rzr = zp.tile([1, 512], F32R, tag="rzr", name="rzr")
                    nc.vector.tensor_copy(rzr[0:1, 0:ln2], rzf[0:1, 0:ln2])
                    przb = pc3.tile([128, 512], F32, tag="przb", name="przb")
                    nc.tensor.matmul(
                        przb[0:64, 0:ln2], ones64[:], rzr[0:1, 0:ln2],
                        start=True, stop=True)
                    rzb = rzp.tile([64, 512], F32, tag="rzb", name="rzb")
                    nc.vector.tensor_copy(rzb[:, 0:ln2], przb[0:64, 0:ln2])
                    nc.gpsimd.tensor_mul(
                        ctxap2[base:base + 64, loff2:loff2 + ln2],
                        ctxu[0:64, 0:ln2], rzb[0:64, 0:ln2])

                BLOCKS = [(2, 0), (2, 1), (2, 2),
                          (0, 0), (1, 0), (0, 1), (1, 1), (0, 2), (1, 2)]
                for bn, (h, lt_i) in enumerate(BLOCKS):
                    loff, ln = L_TILES[lt_i]
                    ktap, qtap = KT_OF_H[h], qt_h[h]
                    ctxap, cbase = CTX_OF_H[h]
                    pctx = pc3.tile([65, 512], F32, tag="pctx")
                    PAIRS = [(i, i + 1) if i + 1 < len(MC) else (i,)
                             for i in range(0, len(MC), 2)]
                    exs = {}

                    def scores_exp(p):
                        chunks = PAIRS[p]
                        ps = ps3.tile([128, 1024], F32, tag="ps", name="ps")
                        for j, i in enumerate(chunks):
                            moff, m = MC[i]
                            nc.tensor.matmul(
                                ps[0:m, j * 512:j * 512 + ln],
                                ktap[0:128, moff:moff + m],
                                qtap[0:128, loff:loff + ln],
                                start=True, stop=True)
                        ex = ep.tile([128, 1024], BF16, tag="ex", name="ex")
                        m0 = MC[chunks[0]][1]
                        if len(chunks) == 2:
                            nc.scalar.activation(
                                ex[0:m0, 0:2 * ln].rearrange(
                                    "p (two n) -> p two n", two=2),
                                ps[0:m0, :].rearrange(
                                    "p (two n) -> p two n", two=2)[:, :, 0:ln],
                                AF.Exp, bias=0.0, scale=SCALE)
                        else:
                            nc.scalar.activation(
                                ex[0:m0, 0:ln], ps[0:m0, 0:ln], AF.Exp,
                                bias=0.0, scale=SCALE)
                        exs[p] = ex

                    def ctx_mm(p):
                        ex = exs.pop(p)
                        for j, i in enumerate(PAIRS[p]):
                            moff, m = MC[i]
                            nc.tensor.matmul(
                                pctx[:, 0:ln], v_all[0:m, i, h, :],
                                ex[0:m, j * ln:j * ln + ln],
                                start=(i == 0), stop=(i == len(MC) - 1))

                    for p in range(len(PAIRS) + 2):
                        if p < len(PAIRS):
                            scores_exp(p)
                        if p in (2, 6) and pending:
                            finish_tile(pending.pop(0))
                        if bn < 3 and p in (1, 4, 7) and fillers:
                            kind, foff, fn = fillers.pop(0)
                            if kind == "k":
                                k01_job(foff, fn)
                            else:
                                q01_job(foff, fn)
                        if bn >= 5 and p in (5, 8) and proj_q:
                            proj_slice(*proj_q.pop(0))
                        if p >= 2:
                            ctx_mm(p - 2)

                    zrow = zp.tile([1, 512], F32, tag="zrow", name="zrow",
                                   bufs=4)
                    nc.vector.tensor_copy(zrow[0:1, 0:ln], pctx[64:65, 0:ln])
                    ctxu = rzp.tile([64, 512], F32, tag="ctxu", name="ctxu")
                    nc.vector.tensor_copy(ctxu[:, 0:ln], pctx[0:64, 0:ln])
                    pending.append((zrow, ctxu, ctxap, cbase, loff, ln))
                    if bn == 4:
                        proj_q.extend(PROJ_OF_LT[0])
                    elif bn == 6:
                        proj_q.extend(PROJ_OF_LT[1])
                    elif bn == 8:
                        proj_q.extend(PROJ_OF_LT[2])
                # tail: remaining chains, then projections
                while pending:
                    finish_tile(pending.pop(0))
                while proj_q:
                    proj_slice(*proj_q.pop(0))
            es.close()

    nc.finalize()
    return nc


def _rope_tables():
    dim = D // 2
    freqs = 1.0 / 10000 ** (np.arange(0, dim, 2, dtype=np.float64) / dim)
    t = np.arange(GRID, dtype=np.float64)
    f = np.repeat(np.outer(t, freqs), 2, axis=-1)                  # [48, 32]
    fr = np.broadcast_to(f[:, None, :], (GRID, GRID, dim))
    fc = np.broadcast_to(f[None, :, :], (GRID, GRID, dim))
    full = np.concatenate([fr, fc], axis=-1).reshape(GRID * GRID, D)
    cos = np.ones((SEQ, D), np.float64)
    sin = np.zeros((SEQ, D), np.float64)
    cos[TASK:] = np.cos(full)
    sin[TASK:] = np.sin(full)
    return cos.astype(np.float32), sin.astype(np.float32)


# interleaved basis: row r <-> (slot=(r//32)%2, d=(r%32)+32*(r//64))
_R = np.arange(128)
_DSEL = (_R % 32) + 32 * (_R // 64)
_HSEL = (_R // 32) % 2


def _core_inputs(x, mask, Wqkv, Wproj, bqkv, cos, sin, g, s):
    xT = x.T  # [768, 2320]
    if s == 0:
        perm = None
        xt = np.ascontiguousarray(xT)
    else:
        perm = np.concatenate([np.arange(SQ, SEQ), np.arange(0, SQ)])
        xt = np.ascontiguousarray(np.concatenate([xT[:, SQ:], xT[:, :SQ]], axis=1))
    r0 = 192 * g
    wq = Wqkv[r0:r0 + 192, :].T                  # [768, 192]
    wk = Wqkv[768 + r0:768 + r0 + 192, :].T
    wv = np.ascontiguousarray(Wqkv[1536 + r0:1536 + r0 + 192, :].T)
    # interleaved-basis weight column orders
    wk01 = wk[:, _HSEL * 64 + _DSEL]                         # heads 0,1
    wq01 = wq[:, _HSEL * 64 + _DSEL]
    wkq2 = np.where(_HSEL[None, :] == 0,
                    wk[:, 128 + _DSEL], wq[:, 128 + _DSEL])  # head2 K|Q
    bq = bqkv[r0:r0 + 192]
    bk = bqkv[768 + r0:768 + r0 + 192]
    bias = np.zeros((1, 3, 128), np.float32)
    bias[0, 0] = bk[_HSEL * 64 + _DSEL]
    bias[0, 1] = bq[_HSEL * 64 + _DSEL]
    bias[0, 2] = np.where(_HSEL == 0, bk[128 + _DSEL], bq[128 + _DSEL])
    wp01 = np.ascontiguousarray(Wproj[:, r0:r0 + 128].T)     # [128, 768]
    wp2 = np.ascontiguousarray(Wproj[:, r0 + 128:r0 + 192].T)  # [64, 768]
    cosT, sinT = cos.T, sin.T  # [64, S]
    cil = cosT[_DSEL]                                        # [128, S]
    # sine table stored at the SOURCE rows so both tensor_tensor inputs
    # share a base partition: rows 64:128 feed t2[0:64] (sign -), etc.
    sil = np.empty((128, SEQ), np.float32)
    sil[64:128] = -sinT[_DSEL[0:64]]
    sil[0:64] = sinT[_DSEL[64:128]]
    if perm is not None:
        cil = cil[:, perm]
        sil = sil[:, perm]
    mk = mask.astype(np.float32)
    if perm is not None:
        mk = mk[perm]
    mk = np.concatenate([mk, np.zeros(19 * 128 - SEQ, np.float32)])
    mk = np.ascontiguousarray(mk.reshape(19, 128).T)
    import ml_dtypes
    bf = ml_dtypes.bfloat16
    return {
        "xt": np.ascontiguousarray(
            np.stack([xt[:, i * 580:(i + 1) * 580] for i in range(4)])
        ).astype(bf),
        "wk01": np.ascontiguousarray(wk01).astype(bf),
        "wq01": np.ascontiguousarray(wq01).astype(bf),
        "wkq2": np.ascontiguousarray(wkq2).astype(bf),
        "wv": wv.astype(bf),
        "wp01": wp01, "wp2": wp2,
        "bias": bias.astype(bf),
        "cil": np.ascontiguousarray(cil).astype(bf),
        "sil": np.ascontiguousarray(sil).astype(bf),
        "mk": np.ascontiguousarray(mk),
        "ones64": np.ones((1, 64), np.float32),
        "onesrow": np.ones((1, SEQ), ml_dtypes.bfloat16),
    }


def _run(x, mask, Wqkv, bqkv, Wproj, bproj, trace=False):
    global _prog
    from concourse.bass_utils import run_bass_kernel_spmd
    if _prog is None:
        _prog = _build()
    x = np.asarray(x, np.float32)
    mask = np.asarray(mask)
    Wqkv = np.asarray(Wqkv, np.float32)
    bqkv = np.asarray(bqkv, np.float32)
    Wproj = np.asarray(Wproj, np.float32)
    bproj = np.asarray(bproj, np.float32)
    cos, sin = _rope_tables()
    in_maps = [
        _core_inputs(x, mask, Wqkv, Wproj, bqkv, cos, sin, core // 2, core % 2)
        for core in range(8)
    ]
    res = run_bass_kernel_spmd(_prog, in_maps, list(range(8)), trace=trace)
    acc = np.zeros((SEQ, E), np.float64)
    for core in range(8):
        s = core % 2
        acc[SQ * s:SQ * (s + 1)] += res.results[core]["pout"].astype(np.float64)
    bias_row = bproj.astype(np.float64) + Wproj.astype(np.float64) @ \
        bqkv[1536:2304].astype(np.float64)
    acc += bias_row
    return acc.astype(np.float32), res


def kernel(x, mask, Wqkv, bqkv, Wproj, bproj):
    out, _ = _run(x, mask, Wqkv, bqkv, Wproj, bproj, trace=False)
    return out


# revision 30
# speedup vs baseline: 1.2426x; 1.0230x over previous
"""Multi-head attention (2D-RoPE, masked softmax) on 8 Trainium2 NeuronCores.

Sharding: 4 head-groups (3 heads each) x 2 query-halves (1160 rows each).
Each core computes full attention for its 3 heads over its 1160 query rows
against all 2320 keys, plus its share of the output projection; the host
sums the 8 partial projections and adds the (folded) biases.

v3 design notes:
  - Interleaved head-dim basis: rows [hA d0:32 | hB d0:32 | hA d32:64 |
    hB d32:64]. rotate_half becomes a uniform +-64 partition shift, so a
    2-head rope tile is copy + 3 muls + 1-4 adds on DVE instead of 8 ops.
    Heads (0,1) pack into one K tile (kt01, no zero padding needed -- the
    moving-side qt_h is zero-padded per head instead); head 2 packs its K
    and Q into one projection matmul (rows K|Q), valid since the permuted
    x layout puts this core's own query tokens first.
  - QKV biases fold into the projection matmuls as a K=1 matmul against a
    constant ones-row (stationary = bias row), freeing ScalarE entirely
    for exp; V masking runs on ScalarE as activation(Copy, scale=mask)
    (mask is per-partition there), freeing DVE for rope.
  - Output projection contracts head pairs: ctxn01 holds heads 0,1 in
    rows 0:64/64:128 (no zero padding), head 2 in ctxn2.
  - Emission order: g1 K|Q tiles -> all V tiles -> h2's three attention
    blocks (g0 K/Q tiles woven into their slots) -> h0/h1 blocks with
    deferred 1/Z chains and the previous L-tile's projections woven in.
  - DMA: few large 3D-AP transfers spread across sync/scalar/gpsimd
    queues; ~130 dummy matmuls hold the PE clock at 2.4 GHz through the
    DMA window; partial projections DMA out as bf16.
"""
import sys
if '/opt/trn_rl_repo' not in sys.path:
    sys.path.insert(0, '/opt/trn_rl_repo')
import numpy as np

SEQ, E, NH, D = 2320, 768, 12, 64
GRID, TASK = 48, 16
SQ = SEQ // 2           # query rows per core
HG = 3                  # heads per core
SCALE = D ** -0.5
EC = 6                  # embed chunks of 128
L_TILES = [(0, 512), (512, 512), (1024, 136)]
N_TILES = [(0, 512), (512, 512), (1024, 512), (1536, 512), (2048, 272)]
MC = [(i * 128, min(128, SEQ - i * 128)) for i in range(19)]
PT = [(i * 128, min(128, SQ - i * 128)) for i in range(10)]
WARM = 75               # PE warmup matmuls (N=128 each, ~128ns cold)

_prog = None


def _build():
    from contextlib import ExitStack
    import concourse.mybir as mybir
    import concourse.tile as tile
    from concourse import bacc

    F32, F32R = mybir.dt.float32, mybir.dt.float32r
    BF16 = mybir.dt.bfloat16
    AF = mybir.ActivationFunctionType

    nc = bacc.Bacc('TRN2', target_bir_lowering=False, debug=False, num_devices=8)
    dp = nc.declare_dram_parameter
    xt_d = dp("xt", [4, E, 580], BF16, isOutput=False)
    wk01_d = dp("wk01", [E, 128], BF16, isOutput=False)
    wq01_d = dp("wq01", [E, 128], BF16, isOutput=False)
    wkq2_d = dp("wkq2", [E, 128], BF16, isOutput=False)
    wv_d = dp("wv", [E, 192], BF16, isOutput=False)
    wp01_d = dp("wp01", [128, E], F32R, isOutput=False)
    wp2_d = dp("wp2", [64, E], F32R, isOutput=False)
    bias_d = dp("bias", [1, 3, 128], BF16, isOutput=False)
    cil_d = dp("cil", [128, SEQ], BF16, isOutput=False)
    sil_d = dp("sil", [128, SEQ], BF16, isOutput=False)
    mk_d = dp("mk", [128, 19], F32, isOutput=False)
    on_d = dp("ones64", [1, 64], F32R, isOutput=False)
    onr_d = dp("onesrow", [1, SEQ], BF16, isOutput=False)
    out_d = dp("pout", [SQ, E], BF16, isOutput=True)

    with tile.TileContext(nc) as tc:
        with (
            tc.tile_pool(name="long", bufs=1) as lp,
            tc.tile_pool(name="zp", bufs=2) as zp,
        ):
            kt01 = lp.tile([128, SEQ], BF16, tag="kt01")
            kt2 = lp.tile([128, SEQ], BF16, tag="kt2")
            qt_h = [lp.tile([128, SQ], BF16, tag=f"qt{h}", name=f"qt{h}")
                    for h in range(HG)]
            rk01 = lp.tile([128, SEQ], BF16, tag="rk01")
            rkq2 = lp.tile([128, SEQ], BF16, tag="rkq2")
            rq01 = lp.tile([128, SQ], BF16, tag="rq01")
            v_all = lp.tile([128, 19, HG, 65], BF16, tag="v_all")
            wp01_sb = lp.tile([128, E], F32R, tag="wp01")
            wp2_sb = lp.tile([128, E], F32R, tag="wp2")
            ctxn01 = lp.tile([128, SQ], F32R, tag="ctxn01")
            ctxn2 = lp.tile([128, SQ], F32R, tag="ctxn2")
            ones64 = lp.tile([1, 64], F32R, tag="ones64")
            ones_row = lp.tile([1, SEQ], BF16, tag="ones_row")
            bias_sb = lp.tile([1, 3, 128], BF16, tag="bias")
            mk_sb = lp.tile([128, 19], F32, tag="mk")
            cil_sb = lp.tile([128, SEQ], BF16, tag="cil")
            sil_sb = lp.tile([128, SEQ], BF16, tag="sil")
            warm_sb = lp.tile([128, 128], BF16, tag="warm")

            # ---- input DMA: few big transfers, spread across queues ----
            nc.gpsimd.memset(warm_sb[:], 0.0)
            nc.gpsimd.dma_start(cil_sb[:], cil_d[:])
            nc.gpsimd.dma_start(sil_sb[:], sil_d[:])
            nc.gpsimd.dma_start(mk_sb[:], mk_d[:])
            nc.gpsimd.dma_start(ones64[:], on_d[:])
            nc.gpsimd.dma_start(ones_row[:], onr_d[:])
            nc.gpsimd.memset(kt2[:], 0.0)
            for h in range(HG):
                nc.gpsimd.memset(qt_h[h][:], 0.0)
            nc.gpsimd.memset(ctxn2[64:128, :].bitcast(F32), 0.0)
            nc.gpsimd.memset(wp2_sb[64:128, :].bitcast(F32), 0.0)

            es = ExitStack()
            p12 = es.enter_context(tc.tile_pool(name="p12", bufs=1))
            pvst = ExitStack()
            pkp0 = pvst.enter_context(tc.tile_pool(name="pkA", bufs=3, space="PSUM"))
            pvp = pvst.enter_context(tc.tile_pool(name="pv", bufs=2, space="PSUM"))
            pkp = [pkp0]

            wk_sb = p12.tile([128, EC, 128], BF16, tag="wk")
            wq_sb = p12.tile([128, EC, 128], BF16, tag="wq")
            wkq_sb = p12.tile([128, EC, 128], BF16, tag="wkq")
            wv_sb = p12.tile([128, EC, 192], BF16, tag="wv")
            xt = p12.tile([128, EC, SEQ], BF16, tag="xt")
            # sync queue: big operands in first-use order
            nc.sync.dma_start(wkq_sb[:, :, :],
                              wkq2_d.rearrange("(c p) n -> p c n", p=128))
            nc.sync.dma_start(xt[:, :, 0:580],
                              xt_d[0].rearrange("(c p) n -> p c n", p=128))
            nc.sync.dma_start(xt[:, :, 580:1160],
                              xt_d[1].rearrange("(c p) n -> p c n", p=128))
            nc.sync.dma_start(xt[:, :, 1160:1740],
                              xt_d[2].rearrange("(c p) n -> p c n", p=128))
            # scalar queue: small/late tensors
            nc.scalar.dma_start(bias_sb[:], bias_d[:])
            nc.scalar.dma_start(wv_sb[:, :, :],
                                wv_d.rearrange("(c p) n -> p c n", p=128))
            nc.scalar.dma_start(xt[:, :, 1740:2320],
                                xt_d[3].rearrange("(c p) n -> p c n", p=128))
            nc.scalar.dma_start(wk_sb[:, :, :],
                                wk01_d.rearrange("(c p) n -> p c n", p=128))
            nc.scalar.dma_start(wq_sb[:, :, :],
                                wq01_d.rearrange("(c p) n -> p c n", p=128))
            nc.scalar.dma_start(wp01_sb[:], wp01_d[:])
            nc.scalar.dma_start(wp2_sb[0:64, :], wp2_d[:])

            # ---- PE warmup: hold HAM at 2.4GHz through the DMA window
            warm_ps = pkp0.tile([128, 512], F32, tag="pk", name="warm_ps")
            for _ in range(WARM):
                nc.tensor.matmul(warm_ps[0:128, 0:128], warm_sb[:],
                                 warm_sb[:], start=True, stop=True)

            # ---- QKV projection tiles (bias folded as K=1 matmul) ----
            def qk_mm(w_sb, bidx, off, n):
                ps = pkp[0].tile([128, 512], F32, tag="pk", name="pk")
                for c in range(EC):
                    nc.tensor.matmul(
                        ps[0:128, 0:n], w_sb[:, c, :], xt[:, c, off:off + n],
                        start=(c == 0), stop=False)
                nc.tensor.matmul(
                    ps[0:128, 0:n], bias_sb[0:1, bidx, :],
                    ones_row[0:1, off:off + n], start=False, stop=True)
                return ps

            # ---- rope in the interleaved basis ----
            def rope_il(ps, raw, off, n, kind):
                if kind == "kq2":
                    nc.scalar.copy(raw[0:128, off:off + n], ps[0:128, 0:n])
                else:
                    nc.vector.tensor_copy(raw[0:128, off:off + n], ps[0:128, 0:n])
                t1 = zp.tile([128, 512], BF16, tag="rt1", name="rt1", bufs=3)
                t2 = zp.tile([128, 512], BF16, tag="rt2", name="rt2", bufs=3)
                nc.vector.tensor_mul(
                    t1[0:128, 0:n], raw[0:128, off:off + n],
                    cil_sb[0:128, off:off + n])
                nc.vector.tensor_mul(
                    t2[0:64, 0:n], raw[64:128, off:off + n],
                    sil_sb[64:128, off:off + n])
                nc.vector.tensor_mul(
                    t2[64:128, 0:n], raw[0:64, off:off + n],
                    sil_sb[0:64, off:off + n])
                if kind == "k01":
                    nc.vector.tensor_add(
                        kt01[0:128, off:off + n], t1[0:128, 0:n], t2[0:128, 0:n])
                elif kind == "q01":
                    nc.vector.tensor_add(
                        qt_h[0][0:32, off:off + n], t1[0:32, 0:n], t2[0:32, 0:n])
                    nc.vector.tensor_add(
                        qt_h[0][64:96, off:off + n], t1[64:96, 0:n], t2[64:96, 0:n])
                    nc.vector.tensor_add(
                        qt_h[1][32:64, off:off + n], t1[32:64, 0:n], t2[32:64, 0:n])
                    nc.vector.tensor_add(
                        qt_h[1][96:128, off:off + n], t1[96:128, 0:n],
                        t2[96:128, 0:n])
                else:  # kq2: K rows stay, Q rows shift up 32
                    nc.vector.tensor_add(
                        kt2[0:32, off:off + n], t1[0:32, 0:n], t2[0:32, 0:n])
                    nc.vector.tensor_add(
                        kt2[64:96, off:off + n], t1[64:96, 0:n], t2[64:96, 0:n])
                    nq = min(n, SQ - off)
                    if nq > 0:
                        nc.vector.tensor_add(
                            qt_h[2][0:32, off:off + nq], t1[32:64, 0:nq],
                            t2[32:64, 0:nq])
                        nc.vector.tensor_add(
                            qt_h[2][64:96, off:off + nq], t1[96:128, 0:nq],
                            t2[96:128, 0:nq])

            def k01_job(off, n):
                rope_il(qk_mm(wk_sb, 0, off, n), rk01, off, n, "k01")

            def q01_job(off, n):
                rope_il(qk_mm(wq_sb, 1, off, n), rq01, off, n, "q01")

            def kq2_job(off, n):
                rope_il(qk_mm(wkq_sb, 2, off, n), rkq2, off, n, "kq2")

            def v_tile(i):
                off, m = MC[i]
                pv = pvp.tile([128, 192], F32, tag="pv", name="pv")
                for c in range(EC):
                    nc.tensor.matmul(
                        pv[0:m, :], xt[:, c, off:off + m], wv_sb[:, c, :],
                        start=(c == 0), stop=(c == EC - 1))
                if i % 2 == 0:
                    nc.scalar.activation(
                        v_all[0:m, i, :, 0:64],
                        pv[0:m, 0:192].rearrange("p (h d) -> p h d", h=HG),
                        AF.Copy, bias=0.0, scale=mk_sb[0:m, i:i + 1])
                else:
                    nc.vector.tensor_mul(
                        v_all[0:m, i, :, 0:64],
                        pv[0:m, 0:192].rearrange("p (h d) -> p h d", h=HG),
                        mk_sb[0:m, i:i + 1].to_broadcast([m, HG, 64]))

            # ones-column plane of v_all (static: per-key mask value)
            for h in range(HG):
                nc.vector.tensor_copy(
                    v_all[:, :, h, 64:65],
                    mk_sb[:, 0:19].to_broadcast([128, 19, 1]))
            # phase A head: g1 K|Q tiles back-to-back (their ropes gate the
            # first exp), then all V tiles (masking on DVE trails into the
            # first attention block)
            for (off, n) in N_TILES:
                kq2_job(off, n)
            for i in range(len(MC)):
                v_tile(i)
            pvst.close()

            # fillers woven into h2's attention blocks
            fillers = [("k", off, n) for (off, n) in N_TILES]
            fillers += [("q", off, n) for (off, n) in L_TILES]

            # ---- attention ----
            with tc.tile_pool(name="ep", bufs=3) as ep, \
                 tc.tile_pool(name="op", bufs=2) as op, \
                 tc.tile_pool(name="rzp", bufs=4) as rzp, \
                 tc.tile_pool(name="ps3", bufs=2, space="PSUM") as ps3, \
                 tc.tile_pool(name="pc3", bufs=1, space="PSUM") as pc3:
                pkp[0] = pc3
                PROJ_OF_LT = {0: PT[0:4], 1: PT[4:8], 2: PT[8:10]}
                KT_OF_H = {0: kt01, 1: kt01, 2: kt2}
                CTX_OF_H = {0: (ctxn01, 0), 1: (ctxn01, 64), 2: (ctxn2, 0)}
                pending = []
                proj_q = []

                def proj_slice(toff, tm):
                    outsb = op.tile([128, E], BF16, tag="outsb", name="outsb")
                    for half in range(2):
                        hs = half * 384
                        pp = pc3.tile([128, 512], F32,
                                      tag=("pp" if half == 0 else "przb"),
                                      name="pp")
                        nc.tensor.matmul(
                            pp[0:tm, 0:384], ctxn01[0:128, toff:toff + tm],
                            wp01_sb[0:128, hs:hs + 384], start=True, stop=False)
                        nc.tensor.matmul(
                            pp[0:tm, 0:384], ctxn2[0:128, toff:toff + tm],
                            wp2_sb[0:128, hs:hs + 384], start=False, stop=True)
                        nc.vector.tensor_copy(outsb[0:tm, hs:hs + 384],
                                              pp[0:tm, 0:384])
                    nc.gpsimd.dma_start(out_d[toff:toff + tm, :], outsb[0:tm, :])

                def finish_tile(z):
                    zrow, ctxu, ctxap2, base, loff2, ln2 = z
                    zscr = zp.tile([1, 512], F32, tag="zscr", name="zscr")
                    rzf = zp.tile([1, 512], F32, tag="rzf", name="rzf")
                    nc.vector.reciprocal_approx_accurate(
                        rzf[0:1, 0:ln2], zrow[0:1, 0:ln2], zscr[0:1, 0:ln2])
                    rzr = zp.tile([1, 512], F32R, tag="rzr", name="rzr")
                    nc.vector.tensor_copy(rzr[0:1, 0:ln2], rzf[0:1, 0:ln2])
                    przb = pc3.tile([128, 512], F32, tag="przb", name="przb")
                    nc.tensor.matmul(
                        przb[0:64, 0:ln2], ones64[:], rzr[0:1, 0:ln2],
                        start=True, stop=True)
                    rzb = rzp.tile([64, 512], F32, tag="rzb", name="rzb")
                    nc.vector.tensor_copy(rzb[:, 0:ln2], przb[0:64, 0:ln2])
                    nc.vector.tensor_mul(
                        ctxap2[base:base + 64, loff2:loff2 + ln2],
                        ctxu[0:64, 0:ln2], rzb[0:64, 0:ln2])

                BLOCKS = [(2, 0), (2, 1), (2, 2),
                          (0, 0), (1, 0), (0, 1), (1, 1), (0, 2), (1, 2)]
                for bn, (h, lt_i) in enumerate(BLOCKS):
                    loff, ln = L_TILES[lt_i]
                    ktap, qtap = KT_OF_H[h], qt_h[h]
                    ctxap, cbase = CTX_OF_H[h]
                    pctx = pc3.tile([65, 512], F32, tag="pctx")
                    PAIRS = [(i, i + 1) if i + 1 < len(MC) else (i,)
                             for i in range(0, len(MC), 2)]
                    exs = {}

                    def scores_exp(p):
                        chunks = PAIRS[p]
                        ps = ps3.tile([128, 1024], F32, tag="ps", name="ps")
                        for j, i in enumerate(chunks):
                            moff, m = MC[i]
                            nc.tensor.matmul(
                                ps[0:m, j * 512:j * 512 + ln],
                                ktap[0:128, moff:moff + m],
                                qtap[0:128, loff:loff + ln],
                                start=True, stop=True)
                        ex = ep.tile([128, 1024], BF16, tag="ex", name="ex")
                        m0 = MC[chunks[0]][1]
                        if len(chunks) == 2:
                            nc.scalar.activation(
                                ex[0:m0, 0:2 * ln].rearrange(
                                    "p (two n) -> p two n", two=2),
                                ps[0:m0, :].rearrange(
                                    "p (two n) -> p two n", two=2)[:, :, 0:ln],
                                AF.Exp, bias=0.0, scale=SCALE)
                        else:
                            nc.scalar.activation(
                                ex[0:m0, 0:ln], ps[0:m0, 0:ln], AF.Exp,
                                bias=0.0, scale=SCALE)
                        exs[p] = ex

                    def ctx_mm(p):
                        ex = exs.pop(p)
                        for j, i in enumerate(PAIRS[p]):
                            moff, m = MC[i]
                            nc.tensor.matmul(
                                pctx[:, 0:ln], v_all[0:m, i, h, :],
                                ex[0:m, j * ln:j * ln + ln],
                                start=(i == 0), stop=(i == len(MC) - 1))

                    for p in range(len(PAIRS) + 2):
                        if p < len(PAIRS):
                            scores_exp(p)
                        if p in (2, 6) and pending:
                            finish_tile(pending.pop(0))
                        if bn < 3 and p in (1, 4, 7) and fillers:
                            kind, foff, fn = fillers.pop(0)
                            if kind == "k":
                                k01_job(foff, fn)
                            else:
                                q01_job(foff, fn)
                        if bn >= 5 and p in (5, 8) and proj_q:
                            proj_slice(*proj_q.pop(0))
                        if p >= 2:
                            ctx_mm(p - 2)

                    zrow = zp.tile([1, 512], F32, tag="zrow", name="zrow",
                                   bufs=4)
                    nc.vector.tensor_copy(zrow[0:1, 0:ln], pctx[64:65, 0:ln])
                    ctxu = rzp.tile([64, 512], F32, tag="ctxu", name="ctxu")
                    nc.vector.tensor_copy(ctxu[:, 0:ln], pctx[0:64, 0:ln])
                    pending.append((zrow, ctxu, ctxap, cbase, loff, ln))
                    if bn == 4:
                        proj_q.extend(PROJ_OF_LT[0])
                    elif bn == 6:
                        proj_q.extend(PROJ_OF_LT[1])
                    elif bn == 8:
                        proj_q.extend(PROJ_OF_LT[2])
                # tail: remaining chains, then projections
                while pending:
                    finish_tile(pending.pop(0))
                while proj_q:
                    proj_slice(*proj_q.pop(0))
            es.close()

    nc.finalize()
    return nc


def _rope_tables():
    dim = D // 2
    freqs = 1.0 / 10000 ** (np.arange(0, dim, 2, dtype=np.float64) / dim)
    t = np.arange(GRID, dtype=np.float64)
    f = np.repeat(np.outer(t, freqs), 2, axis=-1)                  # [48, 32]
    fr = np.broadcast_to(f[:, None, :], (GRID, GRID, dim))
    fc = np.broadcast_to(f[None, :, :], (GRID, GRID, dim))
    full = np.concatenate([fr, fc], axis=-1).reshape(GRID * GRID, D)
    cos = np.ones((SEQ, D), np.float64)
    sin = np.zeros((SEQ, D), np.float64)
    cos[TASK:] = np.cos(full)
    sin[TASK:] = np.sin(full)
    return cos.astype(np.float32), sin.astype(np.float32)


# interleaved basis: row r <-> (slot=(r//32)%2, d=(r%32)+32*(r//64))
_R = np.arange(128)
_DSEL = (_R % 32) + 32 * (_R // 64)
_HSEL = (_R // 32) % 2


def _core_inputs(x, mask, Wqkv, Wproj, bqkv, cos, sin, g, s):
    xT = x.T  # [768, 2320]
    if s == 0:
        perm = None
        xt = np.ascontiguousarray(xT)
    else:
        perm = np.concatenate([np.arange(SQ, SEQ), np.arange(0, SQ)])
        xt = np.ascontiguousarray(np.concatenate([xT[:, SQ:], xT[:, :SQ]], axis=1))
    r0 = 192 * g
    wq = Wqkv[r0:r0 + 192, :].T                  # [768, 192]
    wk = Wqkv[768 + r0:768 + r0 + 192, :].T
    wv = np.ascontiguousarray(Wqkv[1536 + r0:1536 + r0 + 192, :].T)
    # interleaved-basis weight column orders
    wk01 = wk[:, _HSEL * 64 + _DSEL]                         # heads 0,1
    wq01 = wq[:, _HSEL * 64 + _DSEL]
    wkq2 = np.where(_HSEL[None, :] == 0,
                    wk[:, 128 + _DSEL], wq[:, 128 + _DSEL])  # head2 K|Q
    bq = bqkv[r0:r0 + 192]
    bk = bqkv[768 + r0:768 + r0 + 192]
    bias = np.zeros((1, 3, 128), np.float32)
    bias[0, 0] = bk[_HSEL * 64 + _DSEL]
    bias[0, 1] = bq[_HSEL * 64 + _DSEL]
    bias[0, 2] = np.where(_HSEL == 0, bk[128 + _DSEL], bq[128 + _DSEL])
    wp01 = np.ascontiguousarray(Wproj[:, r0:r0 + 128].T)     # [128, 768]
    wp2 = np.ascontiguousarray(Wproj[:, r0 + 128:r0 + 192].T)  # [64, 768]
    cosT, sinT = cos.T, sin.T  # [64, S]
    cil = cosT[_DSEL]                                        # [128, S]
    # sine table stored at the SOURCE rows so both tensor_tensor inputs
    # share a base partition: rows 64:128 feed t2[0:64] (sign -), etc.
    sil = np.empty((128, SEQ), np.float32)
    sil[64:128] = -sinT[_DSEL[0:64]]
    sil[0:64] = sinT[_DSEL[64:128]]
    if perm is not None:
        cil = cil[:, perm]
        sil = sil[:, perm]
    mk = mask.astype(np.float32)
    if perm is not None:
        mk = mk[perm]
    mk = np.concatenate([mk, np.zeros(19 * 128 - SEQ, np.float32)])
    mk = np.ascontiguousarray(mk.reshape(19, 128).T)
    import ml_dtypes
    bf = ml_dtypes.bfloat16
    return {
        "xt": np.ascontiguousarray(
            np.stack([xt[:, i * 580:(i + 1) * 580] for i in range(4)])
        ).astype(bf),
        "wk01": np.ascontiguousarray(wk01).astype(bf),
        "wq01": np.ascontiguousarray(wq01).astype(bf),
        "wkq2": np.ascontiguousarray(wkq2).astype(bf),
        "wv": wv.astype(bf),
        "wp01": wp01, "wp2": wp2,
        "bias": bias.astype(bf),
        "cil": np.ascontiguousarray(cil).astype(bf),
        "sil": np.ascontiguousarray(sil).astype(bf),
        "mk": np.ascontiguousarray(mk),
        "ones64": np.ones((1, 64), np.float32),
        "onesrow": np.ones((1, SEQ), ml_dtypes.bfloat16),
    }


def _run(x, mask, Wqkv, bqkv, Wproj, bproj, trace=False):
    global _prog
    from concourse.bass_utils import run_bass_kernel_spmd
    if _prog is None:
        _prog = _build()
    x = np.asarray(x, np.float32)
    mask = np.asarray(mask)
    Wqkv = np.asarray(Wqkv, np.float32)
    bqkv = np.asarray(bqkv, np.float32)
    Wproj = np.asarray(Wproj, np.float32)
    bproj = np.asarray(bproj, np.float32)
    cos, sin = _rope_tables()
    in_maps = [
        _core_inputs(x, mask, Wqkv, Wproj, bqkv, cos, sin, core // 2, core % 2)
        for core in range(8)
    ]
    res = run_bass_kernel_spmd(_prog, in_maps, list(range(8)), trace=trace)
    acc = np.zeros((SEQ, E), np.float64)
    for core in range(8):
        s = core % 2
        acc[SQ * s:SQ * (s + 1)] += res.results[core]["pout"].astype(np.float64)
    bias_row = bproj.astype(np.float64) + Wproj.astype(np.float64) @ \
        bqkv[1536:2304].astype(np.float64)
    acc += bias_row
    return acc.astype(np.float32), res


def kernel(x, mask, Wqkv, bqkv, Wproj, bproj):
    out, _ = _run(x, mask, Wqkv, bqkv, Wproj, bproj, trace=False)
    return out
